# revision 1
# baseline (speedup 1.0000x reference)
"""Trainium2 Bass kernel for nn_CausalAttention (gated-resnet q/k/v projections
+ causal attention). Data-parallel over batch: 8 batches -> 8 NeuronCores.

Per-core computation (batch b), all fp32 storage:
  x_q = query[b] (C=256, S=1024)   x_k = key[b] (256, 1024)
  branch(p, x): e  = elu(x)
                h1 = W1 @ e + b1 ; e1 = elu(h1)
                h2 = W2 @ e1 + b2 ; a, g = split(h2)
                gr = x + a * sigmoid(g)
                o  = Wn @ gr + bn          (512, 1024) channel-major
  q = branch(q, x_q); k = branch(k, x_k); v = branch(v, x_k)
  att view: X_att[s, d] = X_cm[s//2, (s%2)*512 + d]  (flat reinterpretation)
  per head n (d = 64n..64n+63):
    scoresT[s2, s1] = sum_d K_att[s2,d] Q_att[s1,d]   (s2 causal blocks)
    eT = exp(scoresT/sqrt(512)) with strict-lower mask (s2 < s1)
    outT[vs, s1] = sum_s2 V_att[s2, 64n+vs] * eT[s2, s1] ; l[s1] = sum_s2 eT
    final[64n+vs, s1] = outT[vs, s1] / l[s1]   (row 0 of l patched to 1)
"""

import os
import sys
import numpy as np

sys.path.insert(0, "/opt/trn_rl_repo")

C = 256
S = 1024
D = 512
NH = 8
KS = 64
VS = 64
SCALE = 1.0 / float(np.sqrt(512.0))
N_CORES = 8

# config knobs (tweaked during optimization)
CFG = {
    "mm_dtype": "bfloat16",  # "float32" | "float32r" | "bfloat16"
    "elu_combine_engine": "vector",  # (e-1)+r
    "gr_add_engine": "vector",       # gr = u + x
    "mask_engine": "vector",         # eT diag *= mask01
    "stop_after": None,              # None | "proj" | "scores" | "pv"
}


def _split_psum_ranges(a, b, max_n=512):
    """Split [a, b) psum column range into chunks that don't cross 512-col
    bank boundaries and are <= max_n wide."""
    out = []
    while a < b:
        nxt = min(b, ((a // 512) + 1) * 512, a + max_n)
        out.append((a, nxt))
        a = nxt
    return out


def build_program(cfg=CFG):
    from contextlib import ExitStack

    import concourse.bacc as bacc
    import concourse.bass as bass
    import concourse.tile as tile
    from concourse import mybir
    from concourse.alu_op_type import AluOpType as Op

    f32 = mybir.dt.float32
    mmdt = getattr(mybir.dt, cfg["mm_dtype"])
    mdt = mmdt  # dtype for matmul-operand SBUF tiles (producers must round)
    AF = mybir.ActivationFunctionType

    nc = bacc.Bacc("TRN2", target_bir_lowering=False, debug=False,
                   num_devices=N_CORES)

    # ---------------- DRAM parameters ----------------
    idt = mybir.dt.bfloat16 if cfg["mm_dtype"] == "bfloat16" else f32
    query = nc.dram_tensor("query", [C, S], idt, kind="ExternalInput").ap()
    key = nc.dram_tensor("key", [C, S], idt, kind="ExternalInput").ap()
    wT = {}
    bias = {}
    wdt = mdt if mdt == mybir.dt.bfloat16 else f32
    for p in ("q", "k", "v"):
        wT[p, 1] = nc.dram_tensor(f"{p}_w1T", [C, C], wdt, kind="ExternalInput").ap()
        wT[p, 2] = nc.dram_tensor(f"{p}_w2T", [C, 2 * C], wdt, kind="ExternalInput").ap()
        wT[p, "n"] = nc.dram_tensor(f"{p}_wnT", [C, D], wdt, kind="ExternalInput").ap()
        bias[p, 1] = nc.dram_tensor(f"{p}_b1", [C], f32, kind="ExternalInput").ap()
        bias[p, 2] = nc.dram_tensor(f"{p}_b2", [2 * C], f32, kind="ExternalInput").ap()
        bias[p, "n"] = nc.dram_tensor(f"{p}_bn", [D], f32, kind="ExternalInput").ap()
    out_d = nc.dram_tensor("out", [D, S], f32, kind="ExternalOutput").ap()

    def eng(name):
        return getattr(nc, name)

    with tile.TileContext(nc) as tc, ExitStack() as ctx:
        # ------------- persistent pools -------------
        persist = ctx.enter_context(tc.tile_pool(name="persist", bufs=1))
        psum_main = ctx.enter_context(tc.tile_pool(name="psum_main", bufs=3, space="PSUM"))
        psum_pv = ctx.enter_context(tc.tile_pool(name="psum_pv", bufs=2, space="PSUM"))
        dram_pool = ctx.enter_context(tc.tile_pool(name="dram", bufs=1, space="DRAM"))

        # persistent tiles
        xq = persist.tile([128, 2, S], idt)
        xk = persist.tile([128, 2, S], idt)
        eluq = persist.tile([128, 2, S], mdt)
        eluk = persist.tile([128, 2, S], mdt)
        qT_m = persist.tile([128, 4, S], mdt)   # Q^T_att: [dd%128, dd//128, s]
        kT_m = persist.tile([128, 4, S], mdt)
        v_aug = persist.tile([128, 8, NH, VS + 1], mdt)  # [s%128, s//128, n, vs|1]
        mask01 = persist.tile([128, 128], mdt)  # [t2, t1] = 1.0 if t1 > t2 else 0

        vproj_dram = dram_pool.tile([D, S], mdt)
        recip_dram = dram_pool.tile([128, 64], f32)

        # PE warm-up: ~18 back-to-back matmuls on scratch data (runs during
        # the input DMA phase; output never read)
        warm = persist.tile([128, 512], mdt, name="warm")
        nc.vector.memset(warm, 0.5)
        wps = psum_main.tile([128, 1024], f32, tag="pm", name="wps")
        for _ in range(18):
            nc.tensor.matmul(wps[:, 0:512], lhsT=warm[:, 0:128],
                             rhs=warm, start=True, stop=True)

        # inputs
        for cc in range(2):
            nc.sync.dma_start(out=xq[:, cc, :], in_=query[cc * 128:(cc + 1) * 128, :])
            nc.sync.dma_start(out=xk[:, cc, :], in_=key[cc * 128:(cc + 1) * 128, :])
        bnb = {"q": persist.tile([128, D], f32, name="bnb_q"),
               "k": persist.tile([128, D], f32, name="bnb_k")}
        for p in ("q", "k"):
            bn_ap = bias[p, "n"]
            bn_bcast = bass.AP(tensor=bn_ap.tensor, offset=bn_ap.offset,
                               ap=[[0, 128]] + list(bn_ap.ap))
            nc.sync.dma_start(out=bnb[p], in_=bn_bcast)
        # strict-lower mask: keep 1.0 where t1 - t2 - 1 >= 0
        nc.gpsimd.memset(mask01, 1.0)
        nc.gpsimd.affine_select(
            out=mask01, in_=mask01, compare_op=Op.is_ge, fill=0.0,
            base=-1, pattern=[[1, 128]], channel_multiplier=-1,
        )

        def elu_from_sbuf(src3, dst3, work):
            """dst = elu(src) for (128, 2, S) sbuf tiles."""
            for cc in range(2):
                r = work.tile([128, S], mdt, tag="wk")
                e = work.tile([128, S], mdt, tag="wk")
                me = work.tile([128, S], mdt, tag="wk")
                nc.vector.tensor_scalar_max(r, src3[:, cc, :], 0.0)
                nc.scalar.activation(e, src3[:, cc, :], AF.Exp)
                nc.vector.tensor_scalar_min(me, e, 1.0)
                eng(cfg["elu_combine_engine"]).scalar_tensor_tensor(
                    dst3[:, cc, :], me, -1.0, r, Op.add, Op.add)

        def branch(p, x3, elu3, transposed):
            """Gated resnet + nin for branch p. Returns after writing either
            qT_m/kT_m (transposed) or v_sb -> vproj_dram (normal)."""
            wpool = ctx_b.enter_context(tc.tile_pool(name=f"w_{p}", bufs=1))
            work = ctx_b.enter_context(tc.tile_pool(name=f"wk_{p}", bufs=8))
            big = ctx_b.enter_context(tc.tile_pool(name=f"big_{p}", bufs=1))

            w1 = wpool.tile([128, 2, C], mdt)
            w2 = wpool.tile([128, 2, 2 * C], mdt)
            wn = wpool.tile([128, 2, D], mdt)
            def wcast(ap):
                return ap if ap.dtype == mdt else ap.bitcast(mdt)
            for kc in range(2):
                nc.sync.dma_start(out=w1[:, kc, :], in_=wcast(wT[p, 1][kc * 128:(kc + 1) * 128, :]))
                nc.sync.dma_start(out=w2[:, kc, :], in_=wcast(wT[p, 2][kc * 128:(kc + 1) * 128, :]))
                nc.sync.dma_start(out=wn[:, kc, :], in_=wcast(wT[p, "n"][kc * 128:(kc + 1) * 128, :]))
            b1 = wpool.tile([128, 2], f32)
            b2 = wpool.tile([128, 4], f32)
            b2h = wpool.tile([128, 4], f32)
            nc.sync.dma_start(out=b1, in_=bias[p, 1].rearrange("(kc p) -> p kc", p=128))
            nc.sync.dma_start(out=b2, in_=bias[p, 2].rearrange("(kc p) -> p kc", p=128))
            nc.vector.tensor_scalar_mul(b2h, b2, 0.5)
            if not transposed:
                bnv = wpool.tile([128, 4], f32)
                nc.sync.dma_start(out=bnv, in_=bias[p, "n"].rearrange("(kc p) -> p kc", p=128))


            # h1 = W1 @ elu(x) + b1 ; e1 = elu(h1)
            e1 = big.tile([128, 2, S], mdt, tag="e1")
            for mc in range(2):
                ps = psum_main.tile([128, 1024], f32, tag="pm")
                h1 = ps[:, 0:S]
                for nk in range(2):
                    for kc in range(2):
                        nc.tensor.matmul(
                            h1[:, nk * 512:(nk + 1) * 512],
                            lhsT=w1[:, kc, mc * 128:(mc + 1) * 128],
                            rhs=elu3[:, kc, nk * 512:(nk + 1) * 512],
                            start=(kc == 0), stop=(kc == 1))
                r = work.tile([128, S], mdt, tag="wk")
                e = work.tile([128, S], mdt, tag="wk")
                me = work.tile([128, S], mdt, tag="wk")
                nc.vector.tensor_scalar(r, h1, b1[:, mc:mc + 1], 0.0, Op.add, Op.max)
                nc.scalar.activation(e, h1, AF.Exp, bias=b1[:, mc:mc + 1])
                nc.vector.tensor_scalar_min(me, e, 1.0)
                eng(cfg["elu_combine_engine"]).scalar_tensor_tensor(
                    e1[:, mc, :], me, -1.0, r, Op.add, Op.add)

            # h2 = W2 @ e1 + b2 ; gr = x + 0.5(a+b2a)(1+tanh(0.5(g+b2g)))
            gr = big.tile([128, 2, S], mdt, tag="gr")
            for cc in range(2):
                ps_a = psum_main.tile([128, 1024], f32, tag="pm")
                a_raw = ps_a[:, 0:S]
                for nk in range(2):
                    for kc in range(2):
                        nc.tensor.matmul(
                            a_raw[:, nk * 512:(nk + 1) * 512],
                            lhsT=w2[:, kc, cc * 128:(cc + 1) * 128],
                            rhs=e1[:, kc, nk * 512:(nk + 1) * 512],
                            start=(kc == 0), stop=(kc == 1))
                ps_g = psum_main.tile([128, 1024], f32, tag="pm")
                g_raw = ps_g[:, 0:S]
                for nk in range(2):
                    for kc in range(2):
                        nc.tensor.matmul(
                            g_raw[:, nk * 512:(nk + 1) * 512],
                            lhsT=w2[:, kc, (2 + cc) * 128:(3 + cc) * 128],
                            rhs=e1[:, kc, nk * 512:(nk + 1) * 512],
                            start=(kc == 0), stop=(kc == 1))
                ha = work.tile([128, S], mdt, tag="wk")
                tg = work.tile([128, S], mdt, tag="wk")
                u = work.tile([128, S], mdt, tag="wk")
                nc.vector.tensor_scalar(ha, a_raw, b2[:, cc:cc + 1], 0.5, Op.add, Op.mult)
                nc.scalar.activation(tg, g_raw, AF.Tanh,
                                     bias=b2h[:, 2 + cc:3 + cc], scale=0.5)
                nc.vector.scalar_tensor_tensor(u, tg, 1.0, ha, Op.add, Op.mult)
                eng(cfg["gr_add_engine"]).tensor_tensor(
                    gr[:, cc, :], u, x3[:, cc, :], Op.add)

            if transposed:
                # o^T[hw, c_out] accumulated in psum; merged into (d, s) layout:
                # target[dd, 2c+jj] = o^T[jj*512+dd, c]
                tgt = qT_m if p == "q" else kT_m
                for hw_p in (0, 4, 1, 5, 2, 6, 3, 7):
                    ps = psum_main.tile([128, 1024], f32, tag="pm")
                    oT = ps[:, 0:D]
                    for kc in range(2):
                        nc.tensor.matmul(
                            oT,
                            lhsT=gr[:, kc, hw_p * 128:(hw_p + 1) * 128],
                            rhs=wn[:, kc, :],
                            start=(kc == 0), stop=(kc == 1))
                    tp, jj = hw_p % 4, hw_p // 4
                    nc.vector.scalar_tensor_tensor(
                        tgt[:, tp, jj::2], oT, 1.0, bnb[p], Op.mult, Op.add)
            else:
                v_sb = big.tile([128, 4, S], mdt, tag="vsb")
                for mc in range(4):
                    ps = psum_main.tile([128, 1024], f32, tag="pm")
                    vo = ps[:, 0:S]
                    for nk in range(2):
                        for kc in range(2):
                            nc.tensor.matmul(
                                vo[:, nk * 512:(nk + 1) * 512],
                                lhsT=wn[:, kc, mc * 128:(mc + 1) * 128],
                                rhs=gr[:, kc, nk * 512:(nk + 1) * 512],
                                start=(kc == 0), stop=(kc == 1))
                    nc.scalar.activation(v_sb[:, mc, :], vo, AF.Identity,
                                         bias=bnv[:, mc:mc + 1])
                    nc.gpsimd.dma_start(out=vproj_dram[mc * 128:(mc + 1) * 128, :],
                                        in_=v_sb[:, mc, :])
                # v_aug[j][p2, n, u] = V_att[128j+p2, 64n+u]; V_att[s, d] =
                # vproj[s//2, (s%2)*512 + d]. ones in column u=VS.
                # dst partitions p are contiguous; src stream visits
                # (c=64j+p//2, half=p%2, head n, col u) in the same order.
                for j in range(8):
                    src = vproj_dram[64 * j:64 * j + 64, :]
                    src = src.rearrange("c (h n u) -> c h n u", h=2, n=NH)
                    nc.sync.dma_start(out=v_aug[:, j, :, 0:VS], in_=src if src.dtype == mdt else src.bitcast(mdt))
                    nc.vector.memset(v_aug[:, j, :, VS:VS + 1], 1.0)

        # ------- branches: v first (DRAM roundtrip overlaps k/q) ------------
        with ExitStack() as ctx_b:
            elu_from_sbuf(xk, eluk, ctx_b.enter_context(tc.tile_pool(name="wk_in", bufs=8)))
            branch("v", xk, eluk, transposed=False)
        with ExitStack() as ctx_b:
            branch("k", xk, eluk, transposed=True)
        with ExitStack() as ctx_b:
            wk_in2 = ctx_b.enter_context(tc.tile_pool(name="wk_in2", bufs=8))
            elu_from_sbuf(xq, eluq, wk_in2)
            branch("q", xq, eluq, transposed=True)

        # ---------------- attention ----------------
        stop_after = cfg.get("stop_after")
        if stop_after == "proj":
            fin0 = persist.tile([128, S], f32)
            nc.vector.tensor_copy(fin0, qT_m[:, 0, :])
            nc.sync.dma_start(out=out_d[0:128, :], in_=fin0)
            nc.vector.tensor_copy(fin0, kT_m[:, 1, :])
            nc.sync.dma_start(out=out_d[128:256, :], in_=fin0)
            nc.vector.tensor_copy(fin0, v_aug[:, :, :, :].rearrange("p a b c -> p (a b c)")[:, 0:S])
            nc.sync.dma_start(out=out_d[256:384, :], in_=fin0)
            nc.sync.dma_start(out=out_d[384:512, :], in_=fin0)
        attention_on = stop_after not in ("proj",)
        with ExitStack() as ctx_a:
            eT_pool = ctx_a.enter_context(tc.tile_pool(name="eT", bufs=3))
            att_small = ctx_a.enter_context(tc.tile_pool(name="att_small", bufs=3))

            # scores psum groups (each <= 1024 cols = 2 banks)
            GROUPS = [(0,), (1, 7), (2, 6), (3, 5), (4,)]
            G = {}
            off = 0
            for grp in GROUPS:
                for j in grp:
                    G[j] = off
                    off += S - 128 * j
            lbuf = persist.tile([128, 64], f32)    # l rows: hc -> parts [8hc,8hc+8)
            rbuf = persist.tile([128, 64], f32)    # 1/l, same layout
            unnorm_by_hc = {}

            for n in range(NH if attention_on else 0):
                tp, po = n // 2, 64 * (n % 2)
                eT = eT_pool.tile([128, 4608], mdt, tag="eT")
                for grp in GROUPS:
                    glen = sum(S - 128 * j for j in grp)
                    gbase = G[grp[0]]
                    ps = psum_main.tile([128, 1024], f32, tag="pm")
                    for j in grp:
                        off = G[j] - gbase
                        lhsT = kT_m[po:po + 64, tp, 128 * j:128 * (j + 1)]
                        for s1a, s1b in _split_psum_ranges(off, off + (S - 128 * j)):
                            nc.tensor.matmul(
                                ps[:, s1a:s1b],
                                lhsT=lhsT,
                                rhs=qT_m[po:po + 64, tp,
                                         128 * j + (s1a - off):128 * j + (s1b - off)],
                                start=True, stop=True)
                    nc.scalar.activation(eT[:, gbase:gbase + glen],
                                         ps[:, 0:glen], AF.Exp, scale=SCALE)
                    for j in grp:
                        eng(cfg["mask_engine"]).tensor_tensor(
                            eT[:, G[j]:G[j] + 128], eT[:, G[j]:G[j] + 128],
                            mask01, Op.mult)

                if stop_after == "scores":
                    fin1 = att_small.tile([128, 512], f32, tag="fin1")
                    nc.vector.tensor_copy(fin1, eT[:, 0:512])
                    nc.sync.dma_start(out=out_d[64 * (n // 2):64 * (n // 2) + 128,
                                                512 * (n % 2):512 * (n % 2) + 512],
                                      in_=fin1)
                    continue
                for c in range(2):
                    pv = psum_pv.tile([VS + 1, 512], f32, tag="pv")
                    jmax = 3 if c == 0 else 7
                    for j in range(jmax + 1):
                        s1a = max(512 * c, 128 * j)
                        s1b = 512 * (c + 1)
                        nc.tensor.matmul(
                            pv[:, s1a - 512 * c:512],
                            lhsT=v_aug[:, j, n, :],
                            rhs=eT[:, G[j] + (s1a - 128 * j):G[j] + (s1b - 128 * j)],
                            start=(j == 0), stop=(j == jmax))
                    if stop_after == "pv":
                        finp = att_small.tile([VS, 512], f32, tag="finp")
                        nc.vector.tensor_copy(finp, pv[0:VS, :])
                        nc.sync.dma_start(
                            out=out_d[VS * n:VS * (n + 1), 512 * c:512 * (c + 1)],
                            in_=finp)
                        continue
                    if c == 0:
                        nc.vector.memset(pv[VS:VS + 1, 0:1], 1.0)
                    # one copy moves outT and the replicated l rows to SBUF
                    hc = 2 * n + c
                    ul = att_small.tile([VS + 1, 512], f32, tag="ul", bufs=8)
                    nc.vector.tensor_copy(ul, pv)
                    nc.sync.dma_start(out=lbuf[8 * hc:8 * hc + 8, :],
                                      in_=ul[VS:VS + 1, :])
                    unnorm_by_hc[hc] = ul
                if stop_after == "pv":
                    continue
                if n % 2 == 0:
                    continue
                # reciprocal for the head pair (32 lbuf rows, 32-aligned base)
                g = n // 2
                nc.vector.reciprocal(rbuf[32 * g:32 * g + 32, :],
                                     lbuf[32 * g:32 * g + 32, :])
                nc.gpsimd.dma_start(out=recip_dram[32 * g:32 * g + 32, :],
                                    in_=rbuf[32 * g:32 * g + 32, :])
                for nn in (n - 1, n):
                    for c in range(2):
                        rbc = att_small.tile([VS, 512], f32, tag="rbc", bufs=6)
                        fin = att_small.tile([VS, 512], f32, tag="fin", bufs=6)
                        rd = recip_dram.rearrange("a b -> (a b)")[
                            1024 * nn + 512 * c:1024 * nn + 512 * (c + 1)]
                        rsrc = bass.AP(tensor=rd.tensor, offset=rd.offset,
                                       ap=[[0, VS]] + list(rd.ap))
                        nc.gpsimd.dma_start(out=rbc, in_=rsrc)
                        nc.vector.tensor_tensor(
                            fin, unnorm_by_hc.pop(2 * nn + c)[0:VS, :],
                            rbc, Op.mult)
                        nc.scalar.dma_start(
                            out=out_d[VS * nn:VS * (nn + 1),
                                      512 * c:512 * (c + 1)],
                            in_=fin)

    nc.compile()
    return nc


_CACHE = {}


def _get_program(cfg_key=None):
    key = cfg_key or "default"
    if key not in _CACHE:
        _CACHE[key] = build_program(CFG)
    return _CACHE[key]


def make_in_map(inp, b):
    """Per-core input dict for batch b (weights host-transposed/cast)."""
    if CFG["mm_dtype"] == "bfloat16":
        import ml_dtypes
        wt = np.dtype(ml_dtypes.bfloat16)
    else:
        wt = np.float32
    m = {
        "query": np.ascontiguousarray(inp["query"][b].reshape(C, S)).astype(wt),
        "key": np.ascontiguousarray(inp["key"][b].reshape(C, S)).astype(wt),
    }
    for p in ("q", "k", "v"):
        m[f"{p}_w1T"] = np.ascontiguousarray(inp[f"{p}_gr_w1"].T).astype(wt)
        m[f"{p}_w2T"] = np.ascontiguousarray(inp[f"{p}_gr_w2"].T).astype(wt)
        m[f"{p}_wnT"] = np.ascontiguousarray(inp[f"{p}_nin_w"].T).astype(wt)
        m[f"{p}_b1"] = inp[f"{p}_gr_b1"]
        m[f"{p}_b2"] = inp[f"{p}_gr_b2"]
        m[f"{p}_bn"] = inp[f"{p}_nin_b"]
    return m


def kernel(**inputs):
    from concourse.bass_utils import run_bass_kernel_spmd

    nc = _get_program()
    inp = {k: np.asarray(v, dtype=np.float32) for k, v in inputs.items()}

    in_maps = [make_in_map(inp, b) for b in range(N_CORES)]

    trace = bool(int(os.environ.get("BASS_KERNEL_TRACE", "0")))
    res = run_bass_kernel_spmd(nc, in_maps, core_ids=list(range(N_CORES)),
                               trace=trace)
    LAST_RUN["exec_time_ns"] = getattr(res, "exec_time_ns", None)
    LAST_RUN["results"] = res
    out = np.stack([res.results[i]["out"].reshape(D, 32, 32)
                    for i in range(N_CORES)])
    return out.astype(np.float32)


LAST_RUN = {}


if __name__ == "__main__":
    nc = build_program()
    print("compiled OK")

